# revision 15
# baseline (speedup 1.0000x reference)
"""Trainium2 Bass kernel for nn_CholecMetric (segment_reduce).

Per-core (1 clip per NeuronCore, data-parallel over N=8):
  score[h,w] = (sum_p iog_max[p] * Gp[p,h,w]) / (sum_p Gp[p,h,w])
  where iog_max[p] = max_t |Gp_p & Gt_t| / |Gt_t|   (0 where undefined)

Key structure (v2 rewrite):
  - Inputs are host-prechunked so every SWDGE cast-DMA (int32 -> fp16) reads
    one fully contiguous DRAM block and writes one contiguous 2KB run per
    partition: ~128 descriptors/chunk instead of ~4096 (descriptor-gen was
    10us serial on Q7 in the old layout).
  - Intersections: 8 c-slices packed per matmul via block-diagonal trick:
    lhsT = gt[:, 8c x 16t] (128 weight cols -> FWL), rhs = gp[:, 8c x 33p],
    psum [128, 264] accumulated by 64 matmuls total (PE was
    instruction-overhead bound at ~66ns/matmul; 512 -> 64 instructions).
    Only the 8 diagonal 16x33 blocks are useful; they are summed at the end.
    Column 32 of each p-block is a ones slot -> gt_area rides along.
  - cover = sum_p Gp via pairwise fp16 tree on DVE (exact: integer counts),
    reciprocal on ACT; fully hidden under the 35us DMA stream.
  - Tail: diag-block sum -> iogs -> transpose -> iog_max -> w broadcast
    (PE), then num = sum_p w_p*Gp_p as two scalar_tensor_tensor chains
    (DVE fp16 2x + GPSIMD), combine, multiply by 1/cover, DMA out.
"""

import numpy as np

import concourse.bass as bass
import concourse.bacc as bacc
import concourse.tile as tile
from concourse import mybir
from concourse.bass_utils import run_bass_kernel_spmd

N, P, T, H, W = 8, 32, 16, 256, 256
HW = H * W            # 65536
K, C = 128, 512       # hw = k*C + (ch*CHC + c)
NCH = 16              # DMA chunks along c
CHC = C // NCH        # 32 cols per chunk
OCT = 4               # c-slices packed per matmul (block j at partition 32j)
NMM = C // OCT        # 128 matmuls
NCORES = 8

NP_DVE = 21           # planes 0..NP_DVE-1 on DVE STT chain; rest on GPSIMD

F32 = mybir.dt.float32
F16 = mybir.dt.float16
I32 = mybir.dt.int32
ALU = mybir.AluOpType
AF = mybir.ActivationFunctionType


def _swap12(v):
    """Swap the two free dims of a 3-dim AP (partition stays first)."""
    return bass.AP(tensor=v.tensor, offset=v.offset,
                   ap=[v.ap[0], v.ap[2], v.ap[1]])


def build():
    nc = bacc.Bacc("TRN2", target_bir_lowering=False, debug=False,
                   num_devices=1)
    gp_d = nc.dram_tensor("gp", [NCH, K, P, CHC], I32, kind="ExternalInput")
    # gt is c-major within a chunk so a [1,128] weight AP spans 8 c-slices
    gt_d = nc.dram_tensor("gt", [NCH, K, CHC, T], I32, kind="ExternalInput")
    id16_d = nc.dram_tensor("id16", [T, T], F32, kind="ExternalInput")
    id32_d = nc.dram_tensor("id32", [P, P], F32, kind="ExternalInput")
    out_d = nc.dram_tensor("score", [HW], F32, kind="ExternalOutput")

    out_r = out_d.rearrange("(k a b) -> k a b", a=NCH, b=CHC)  # [128,16,32]

    with tile.TileContext(nc) as tc:
        with (
            tc.tile_pool(name="data", bufs=1) as data,
            tc.tile_pool(name="work", bufs=1) as work,
            tc.tile_pool(name="lvl", bufs=2) as lvlp,
            tc.tile_pool(name="small", bufs=1) as small,
            tc.tile_pool(name="psum", bufs=1, space="PSUM") as psum,
        ):
            # chunk-major data tiles: [K, chunk, plane, c-in-chunk]
            gp_t = data.tile([K, NCH, P + 1, CHC], F16, tag="gp")
            gt_t = data.tile([K, NCH, CHC, T], F16, tag="gt")
            # weights with 32-partition-aligned blocks: per chunk, 8 octets
            # of [4 c-slices x 32 slots] (slots 16..31 are zero padding) so
            # each psum diag block starts at partition 32j (HW alignment rule)
            gt_pad = data.tile([K, NCH, CHC // OCT, OCT, 2 * T], F16,
                               tag="gtpad")

            # constants
            id16 = small.tile([T, T], F32, tag="id16")
            id32 = small.tile([P, P], F32, tag="id32")
            ones128 = small.tile([1, K], F32, tag="ones128")
            nc.sync.dma_start(out=id16[:], in_=id16_d[:])
            nc.sync.dma_start(out=id32[:], in_=id32_d[:])
            nc.vector.memset(ones128[:], 1.0)
            # ones slot (p=32) in every chunk, for gt_area via matmul
            nc.vector.memset(gp_t[:, :, P, :], 1.0)

            # zero the pad slots (16..31) of every octet block
            pv = gt_pad[:]
            nc.vector.memset(
                bass.AP(tensor=pv.tensor, offset=pv.offset + T,
                        ap=[pv.ap[0], [2 * T, NCH * CHC], [1, T]]), 0.0)

            # chunked cast loads (SWDGE int32 -> fp16); contiguous DRAM blocks
            for j in range(NCH):
                nc.gpsimd.dma_start(out=gp_t[:, j, 0:P, :], in_=gp_d[j])
                nc.gpsimd.dma_start(out=gt_t[:, j, :, :], in_=gt_d[j])
                # re-layout gt chunk into the padded weight tile: c-flat step
                # 2T, t contiguous (one 4x fp16 DVE copy per chunk)
                src = gt_t[:, j, :, :]
                dv = gt_pad[:, j, :, :, :]
                dst = bass.AP(tensor=dv.tensor, offset=dv.offset,
                              ap=[dv.ap[0], [2 * T, CHC], [1, T]])
                nc.vector.tensor_copy(dst, src)

            # intersections + gt_area: 128 block-diagonal matmuls into one
            # psum bank.  m = j*32+t (j = c-offset in octet), n = p*4+j.
            psum_i = psum.tile([K, OCT * (P + 1)], F32, tag="inters")
            for i in range(NMM):
                ch, oc = i // (CHC // OCT), i % (CHC // OCT)
                lv = gt_pad[:, ch, oc, :, :]
                lhsT = bass.AP(tensor=lv.tensor, offset=lv.offset,
                               ap=[lv.ap[0], [1, K]])
                rhs = gp_t[:, ch, :, OCT * oc:OCT * (oc + 1)]
                nc.tensor.matmul(psum_i[:], lhsT, rhs,
                                 start=(i == 0), stop=(i == NMM - 1))

            # cover = sum_p Gp: pairwise fp16 tree per 2-chunk range on DVE
            # (exact: integer counts <= 32), 1/cover on ACT; all hidden
            # under the DMA stream.
            covm = work.tile([K, NCH, CHC], F32, tag="covm")
            rcov = work.tile([K, NCH, CHC], F32, tag="rcov")
            for r in range(NCH // 2):
                l1 = lvlp.tile([K, 2, 16, CHC], F16, tag="l1")
                l2 = lvlp.tile([K, 2, 8, CHC], F16, tag="l2")
                l3 = lvlp.tile([K, 2, 4, CHC], F16, tag="l3")
                l4 = lvlp.tile([K, 2, 2, CHC], F16, tag="l4")
                base = gp_t[:, 2 * r:2 * r + 2, :, :]
                ap_e = bass.AP(tensor=base.tensor, offset=base.offset,
                               ap=[base.ap[0], base.ap[1], [2 * CHC, 16],
                                   [1, CHC]])
                ap_o = bass.AP(tensor=base.tensor, offset=base.offset + CHC,
                               ap=[base.ap[0], base.ap[1], [2 * CHC, 16],
                                   [1, CHC]])
                nc.vector.tensor_tensor(l1[:], ap_e, ap_o, ALU.add)

                def _half(t_, n_):
                    v = t_[:]
                    e = bass.AP(tensor=v.tensor, offset=v.offset,
                                ap=[v.ap[0], v.ap[1], [2 * CHC, n_], [1, CHC]])
                    o = bass.AP(tensor=v.tensor, offset=v.offset + CHC,
                                ap=[v.ap[0], v.ap[1], [2 * CHC, n_], [1, CHC]])
                    return e, o

                e, o = _half(l1, 8)
                nc.vector.tensor_tensor(l2[:], e, o, ALU.add)
                e, o = _half(l2, 4)
                nc.vector.tensor_tensor(l3[:], e, o, ALU.add)
                e, o = _half(l3, 2)
                nc.vector.tensor_tensor(l4[:], e, o, ALU.add)
                e, o = _half(l4, 1)
                nc.vector.tensor_tensor(covm[:, 2 * r:2 * r + 2, :], e, o,
                                        ALU.add)
                # rcov = 1/max(cover, 0.5): exact for cover>=1; num==0 when
                # cover==0 so the clamp value never shows in the output
                nc.vector.tensor_scalar_max(covm[:, 2 * r:2 * r + 2, :],
                                            covm[:, 2 * r:2 * r + 2, :], 0.5)
                nc.vector.reciprocal(rcov[:, 2 * r:2 * r + 2, :],
                                     covm[:, 2 * r:2 * r + 2, :])

            # ---- tail: diagonal-block sum -> w ----
            # block j lives at partitions [32j,32j+16), cols p*4+j
            def diag(j):
                v = psum_i[32 * j:32 * j + T, :]
                return bass.AP(tensor=v.tensor, offset=v.offset + j,
                               ap=[v.ap[0], [OCT, P + 1]])

            its = small.tile([T, P + 1], F32, tag="its")
            nc.scalar.copy(its[:], diag(0))
            for j in range(1, OCT):
                nc.vector.tensor_tensor(its[:], diag(j), its[:], ALU.add)

            areag = small.tile([T, 1], F32, tag="areag")
            nc.vector.tensor_scalar_max(areag[:], its[:, P:P + 1], 0.5)
            rarea = small.tile([T, 1], F32, tag="rarea")
            nc.vector.reciprocal(rarea[:], areag[:])
            iogs = small.tile([T, P], F32, tag="iogs")
            nc.vector.tensor_scalar_mul(iogs[:], its[:, 0:P], rarea[:, 0:1])
            # transpose iogs -> [P, T], reduce max over t -> iog_max [P, 1]
            psum_tr = psum.tile([P, T], F32, tag="tr")
            nc.tensor.transpose(psum_tr[:], iogs[:], id16[:])
            iomax = small.tile([P, 1], F32, tag="iomax")
            nc.vector.tensor_reduce(iomax[:], psum_tr[:],
                                    mybir.AxisListType.X, ALU.max)
            # w as a row then broadcast to 128 rows via PE
            psum_wr = psum.tile([1, P], F32, tag="wr")
            nc.tensor.matmul(psum_wr[:], iomax[:], id32[:])
            w_row = small.tile([1, P], F32, tag="wrow")
            nc.scalar.copy(w_row[:], psum_wr[:])
            psum_wb = psum.tile([K, P], F32, tag="wb")
            nc.tensor.matmul(psum_wb[:], ones128[:], w_row[:])
            w_bc = small.tile([K, P], F32, tag="wbc")
            nc.vector.tensor_copy(w_bc[:], psum_wb[:])

            # ---- num = sum_p w_p * Gp_p: two fp16 STT chains ----
            acc_d = work.tile([K, NCH, CHC], F16, tag="accd")
            acc_g = work.tile([K, NCH, CHC], F16, tag="accg")
            nc.vector.tensor_scalar_mul(acc_d[:], gp_t[:, :, 0, :],
                                        w_bc[:, 0:1])
            for p in range(1, NP_DVE):
                nc.vector.scalar_tensor_tensor(
                    acc_d[:], gp_t[:, :, p, :], w_bc[:, p:p + 1], acc_d[:],
                    ALU.mult, ALU.add)
            # GPSIMD has no TensorScalarPtr: ACT prescales, GPSIMD adds
            nc.scalar.mul(acc_g[:], gp_t[:, :, NP_DVE, :],
                          w_bc[:, NP_DVE:NP_DVE + 1])
            with tc.tile_pool(name="gtmp", bufs=4) as gtmp_pool:
                for p in range(NP_DVE + 1, P):
                    gtmp = gtmp_pool.tile([K, NCH, CHC], F16, tag="gtmp")
                    nc.scalar.mul(gtmp[:], gp_t[:, :, p, :], w_bc[:, p:p + 1])
                    nc.gpsimd.tensor_tensor(acc_g[:], acc_g[:], gtmp[:],
                                            ALU.add)

            num = work.tile([K, NCH, CHC], F32, tag="num")
            nc.vector.tensor_tensor(num[:], acc_d[:], acc_g[:], ALU.add)
            score = work.tile([K, NCH, CHC], F32, tag="score")
            nc.vector.tensor_tensor(score[:], num[:], rcov[:], ALU.mult)

            nc.sync.dma_start(out=out_r[:], in_=score[:])

    nc.compile()
    return nc


_NC_CACHE = None


def _get_nc():
    global _NC_CACHE
    if _NC_CACHE is None:
        _NC_CACHE = build()
    return _NC_CACHE


def kernel(groups_pred: np.ndarray, groups_true: np.ndarray, trace=False,
           **trace_kwargs) -> np.ndarray:
    nc = _get_nc()
    gp = np.asarray(groups_pred, dtype=np.int32)
    gt = np.asarray(groups_true, dtype=np.int32)
    # hw = k*512 + ch*32 + c  ->  chunk-major contiguous [NCH, K, plane, c]
    gp_c = np.ascontiguousarray(
        gp.reshape(N, P, K, NCH, CHC).transpose(0, 3, 2, 1, 4))
    gt_c = np.ascontiguousarray(
        gt.reshape(N, T, K, NCH, CHC).transpose(0, 3, 2, 4, 1))
    id16 = np.eye(T, dtype=np.float32)
    id32 = np.eye(P, dtype=np.float32)
    in_maps = [{"gp": gp_c[n], "gt": gt_c[n], "id16": id16, "id32": id32}
               for n in range(N)]
    res = run_bass_kernel_spmd(nc, in_maps, list(range(NCORES)), trace=trace,
                               **trace_kwargs)
    out = np.stack([res.results[n]["score"].reshape(H, W) for n in range(N)])
    if trace:
        kernel.last_results = res
    return out.astype(np.float32)


# revision 18
# speedup vs baseline: 1.0330x; 1.0330x over previous
"""Trainium2 Bass kernel for nn_CholecMetric (segment_reduce).

Per-core (1 clip per NeuronCore, data-parallel over N=8):
  score[h,w] = (sum_p iog_max[p] * Gp[p,h,w]) / (sum_p Gp[p,h,w])
  where iog_max[p] = max_t |Gp_p & Gt_t| / |Gt_t|   (0 where undefined)

v3:
  - gp SBUF tile is PLANAR [K, 33, 512] fp16 (contiguous planes -> fast DVE
    tail ops); loaded by 8 SWDGE cast-DMAs of 64 cols each from host
    pre-transposed DRAM [8, K, 33, 64]; plane 32 is a host-side ones plane
    (gt_area rides the matmul, no memset).
  - gt arrives chunk-major contiguous (fast descriptor gen), then DVE/ACT
    copies build gt_pad: per 32-col chunk, 8 octets of [4 c x 32 slots],
    slots 16..31 garbage (ACT copy) so each psum block starts at partition
    32j (HW 32-alignment rule for partition bases).
  - Intersections: 4 c-slices per matmul, 128 block-diagonal matmuls into
    one psum bank [128, 132]; diagonal blocks summed at the end.
  - cover via pairwise fp16 trees on DVE (exact integer counts), hidden
    under the DMA stream; rcov = 1/max(cover,0.5) likewise.
  - Tail: diag sum -> iogs -> PE transpose -> iog_max -> w broadcast (PE);
    num = sum_p w_p*Gp_p as DVE fp16 STT chain + (ACT prescale -> GPSIMD
    add) chain; combine, multiply by rcov, DMA out.
"""

import numpy as np

import concourse.bass as bass
import concourse.bacc as bacc
import concourse.tile as tile
from concourse import mybir
from concourse.bass_utils import run_bass_kernel_spmd

N, P, T, H, W = 8, 32, 16, 256, 256
HW = H * W            # 65536
K, C = 128, 512       # hw = k*C + c
NCH = 8               # DMA chunks along c
CHC = C // NCH        # 64 cols per chunk
OCT = 4               # c-slices packed per matmul (block j at partition 32j)
NMM = C // OCT        # 128 matmuls
NCORES = 8

NP_DVE = 18           # planes 0..NP_DVE-1 on DVE STT chain; rest ACT+GPSIMD

F32 = mybir.dt.float32
F16 = mybir.dt.float16
I32 = mybir.dt.int32
ALU = mybir.AluOpType


def build():
    nc = bacc.Bacc("TRN2", target_bir_lowering=False, debug=False,
                   num_devices=1)
    gp_d = nc.dram_tensor("gp", [NCH, K, P + 1, CHC], I32,
                          kind="ExternalInput")
    gt_d = nc.dram_tensor("gt", [NCH, K, CHC, T], I32, kind="ExternalInput")
    id16_d = nc.dram_tensor("id16", [T, T], F32, kind="ExternalInput")
    id32_d = nc.dram_tensor("id32", [P, P], F32, kind="ExternalInput")
    out_d = nc.dram_tensor("score", [HW], F32, kind="ExternalOutput")

    out_r = out_d.rearrange("(k c) -> k c", c=C)  # [128, 512]

    with tile.TileContext(nc) as tc:
        with (
            tc.tile_pool(name="data", bufs=1) as data,
            tc.tile_pool(name="work", bufs=1) as work,
            tc.tile_pool(name="lvl", bufs=2) as lvlp,
            tc.tile_pool(name="small", bufs=1) as small,
            tc.tile_pool(name="psum", bufs=1, space="PSUM") as psum,
        ):
            gp_t = data.tile([K, P + 1, C], F16, tag="gp")     # planar
            gt_t = data.tile([K, NCH, CHC, T], F16, tag="gt")  # chunk-major
            # weights: [c-slice x 32 slots]; one matmul's lhsT spans 4
            # consecutive c-slices = 128 contiguous weight columns
            gt_pad = data.tile([K, C, 2 * T], F16, tag="gtpad")

            # input DMAs first so SWDGE descriptor gen starts immediately
            for j in range(NCH):
                c0 = j * CHC
                nc.gpsimd.dma_start(out=gp_t[:, :, c0:c0 + CHC], in_=gp_d[j])
                nc.gpsimd.dma_start(out=gt_t[:, j, :, :], in_=gt_d[j])

            id16 = small.tile([T, T], F32, tag="id16")
            id32 = small.tile([P, P], F32, tag="id32")
            ones128 = small.tile([1, K], F32, tag="ones128")
            nc.sync.dma_start(out=id16[:], in_=id16_d[:])
            nc.sync.dma_start(out=id32[:], in_=id32_d[:])
            nc.vector.memset(ones128[:], 1.0)

            # build gt_pad: DVE writes real slots 0..15 (one 4x fp16 copy per
            # chunk), ACT fills pad slots 16..31 with the same data (garbage
            # weights only feed psum rows we never read; avoids a big memset)
            for j in range(NCH):
                src = gt_t[:, j, :, :]
                dv = gt_pad[:, j * CHC:(j + 1) * CHC, :]
                dst_re = bass.AP(tensor=dv.tensor, offset=dv.offset,
                                 ap=[dv.ap[0], [2 * T, CHC], [1, T]])
                nc.vector.tensor_copy(dst_re, src)
                dst_pad = bass.AP(tensor=dv.tensor, offset=dv.offset + T,
                                  ap=[dv.ap[0], [2 * T, CHC], [1, T]])
                nc.scalar.copy(dst_pad, src)

            # intersections + gt_area: 128 block-diagonal matmuls into one
            # psum bank.  m = j*32+t (j = c-offset in octet), n = p*4+j.
            psum_i = psum.tile([K, OCT * (P + 1)], F32, tag="inters")
            for i in range(NMM):
                lv = gt_pad[:, OCT * i:OCT * (i + 1), :]
                lhsT = bass.AP(tensor=lv.tensor, offset=lv.offset,
                               ap=[lv.ap[0], [1, K]])
                rhs = gp_t[:, :, OCT * i:OCT * (i + 1)]
                nc.tensor.matmul(psum_i[:], lhsT, rhs,
                                 start=(i == 0), stop=(i == NMM - 1))

            # cover = sum_p Gp: planar pairwise fp16 trees per c-range on
            # DVE (exact: integer counts), 1/cover on DVE; hidden under DMA
            covm = work.tile([K, C], F32, tag="covm")
            rcov = work.tile([K, C], F32, tag="rcov")
            for r, (r0, r1) in enumerate(((0, 256), (256, 384), (384, 512))):
                w_ = r1 - r0
                lv = gp_t
                nsl, base = P, r0
                lvidx = 0
                while nsl > 2:
                    nxt = lvlp.tile([K, nsl // 2, w_], F16,
                                    tag=f"lv{lvidx}_{w_}")
                    for q in range(nsl // 2):
                        nc.vector.tensor_tensor(
                            nxt[:, q, :], lv[:, 2 * q, base:base + w_],
                            lv[:, 2 * q + 1, base:base + w_], ALU.add)
                    lv, nsl, base, lvidx = nxt, nsl // 2, 0, lvidx + 1
                nc.vector.tensor_tensor(covm[:, r0:r1], lv[:, 0, :],
                                        lv[:, 1, :], ALU.add)
                nc.vector.tensor_scalar_max(covm[:, r0:r1], covm[:, r0:r1],
                                            0.5)
                nc.vector.reciprocal(rcov[:, r0:r1], covm[:, r0:r1])

            # ---- tail: diagonal-block sum -> w ----
            def diag(j):
                v = psum_i[32 * j:32 * j + T, :]
                return bass.AP(tensor=v.tensor, offset=v.offset + j,
                               ap=[v.ap[0], [OCT, P + 1]])

            its = small.tile([T, P + 1], F32, tag="its")
            nc.scalar.copy(its[:], diag(0))
            for j in range(1, OCT):
                nc.vector.tensor_tensor(its[:], diag(j), its[:], ALU.add)

            areag = small.tile([T, 1], F32, tag="areag")
            nc.vector.tensor_scalar_max(areag[:], its[:, P:P + 1], 0.5)
            rarea = small.tile([T, 1], F32, tag="rarea")
            nc.vector.reciprocal(rarea[:], areag[:])
            iogs = small.tile([T, P], F32, tag="iogs")
            nc.vector.tensor_scalar_mul(iogs[:], its[:, 0:P], rarea[:, 0:1])
            psum_tr = psum.tile([P, T], F32, tag="tr")
            nc.tensor.transpose(psum_tr[:], iogs[:], id16[:])
            iomax = small.tile([P, 1], F32, tag="iomax")
            nc.vector.tensor_reduce(iomax[:], psum_tr[:],
                                    mybir.AxisListType.X, ALU.max)
            psum_wr = psum.tile([1, P], F32, tag="wr")
            nc.tensor.matmul(psum_wr[:], iomax[:], id32[:])
            w_row = small.tile([1, P], F32, tag="wrow")
            nc.scalar.copy(w_row[:], psum_wr[:])
            psum_wb = psum.tile([K, P], F32, tag="wb")
            nc.tensor.matmul(psum_wb[:], ones128[:], w_row[:])
            w_bc = small.tile([K, P], F32, tag="wbc")
            nc.vector.tensor_copy(w_bc[:], psum_wb[:])

            # ---- num = sum_p w_p * Gp_p ----
            acc_d = work.tile([K, C], F16, tag="accd")
            acc_g = work.tile([K, C], F16, tag="accg")
            nc.vector.tensor_scalar_mul(acc_d[:], gp_t[:, 0, :], w_bc[:, 0:1])
            for p in range(1, NP_DVE):
                nc.vector.scalar_tensor_tensor(
                    acc_d[:], gp_t[:, p, :], w_bc[:, p:p + 1], acc_d[:],
                    ALU.mult, ALU.add)
            nc.scalar.mul(acc_g[:], gp_t[:, NP_DVE, :],
                          w_bc[:, NP_DVE:NP_DVE + 1])
            with tc.tile_pool(name="gtmp", bufs=4) as gtmp_pool:
                for p in range(NP_DVE + 1, P):
                    gtmp = gtmp_pool.tile([K, C], F16, tag="gtmp")
                    nc.scalar.mul(gtmp[:], gp_t[:, p, :], w_bc[:, p:p + 1])
                    nc.gpsimd.tensor_tensor(acc_g[:], acc_g[:], gtmp[:],
                                            ALU.add)

            num = work.tile([K, C], F32, tag="num")
            nc.vector.tensor_tensor(num[:], acc_d[:], acc_g[:], ALU.add)
            score = work.tile([K, C], F32, tag="score")
            nc.vector.tensor_tensor(score[:], num[:], rcov[:], ALU.mult)

            nc.sync.dma_start(out=out_r[:], in_=score[:])

    nc.compile()
    return nc


_NC_CACHE = None


def _get_nc():
    global _NC_CACHE
    if _NC_CACHE is None:
        _NC_CACHE = build()
    return _NC_CACHE


def kernel(groups_pred: np.ndarray, groups_true: np.ndarray, trace=False,
           **trace_kwargs) -> np.ndarray:
    nc = _get_nc()
    gp = np.asarray(groups_pred, dtype=np.int32)
    gt = np.asarray(groups_true, dtype=np.int32)
    # gp: [N, P, HW] -> per 64-col chunk [NCH, K, P+1, CHC], plane 32 = ones
    gp5 = gp.reshape(N, P, K, NCH, CHC)
    gp_c = np.empty((N, NCH, K, P + 1, CHC), np.int32)
    gp_c[:, :, :, 0:P, :] = gp5.transpose(0, 3, 2, 1, 4)
    gp_c[:, :, :, P, :] = 1
    gp_c = np.ascontiguousarray(gp_c)
    # gt: c-major within chunk [NCH, K, CHC, T]
    gt_c = np.ascontiguousarray(
        gt.reshape(N, T, K, NCH, CHC).transpose(0, 3, 2, 4, 1))
    id16 = np.eye(T, dtype=np.float32)
    id32 = np.eye(P, dtype=np.float32)
    in_maps = [{"gp": gp_c[n], "gt": gt_c[n], "id16": id16, "id32": id32}
               for n in range(N)]
    res = run_bass_kernel_spmd(nc, in_maps, list(range(NCORES)), trace=trace,
                               **trace_kwargs)
    out = np.stack([res.results[n]["score"].reshape(H, W) for n in range(N)])
    if trace:
        kernel.last_results = res
    return out.astype(np.float32)
